# revision 28
# baseline (speedup 1.0000x reference)
"""BottleneckAttn TRN2 kernel.

Reference computation (per batch b, head n, fp32):
    qkv = w_qkv @ x_b                      # (1536, 1024), 1x1 conv
    q, k, v per head: (1024, 128) with hw = h*32 + w
    logits[q,k] = SCALE * (q . k) + qw[q, 31 + w2(k) - w(q)] + qh[q, 31 + h2(k) - h(q)]
        where qw[q,r] = q . width_rel[r], qh[q,r] = q . height_rel[r]
    out = softmax(logits) @ v              # (1024, 128)
    output[b] flat index = q*512 + n*128 + d  -> reshape (512, 32, 32)

Device strategy (SPMD, 8 cores, 2 batches/core):
  - All matmuls computed in the TRANSPOSED softmax layout ST[k, q] so the
    attention probabilities come out of the exp directly in the layout the
    PV matmul needs as its moving operand (no P transposes).
  - ST = k @ qT via PE (operands swapped); the relative-position bias is
    folded in as a second accumulating matmul with a constant 0/1 selection
    matrix lhsT (rows select the shifted width/height tables per PSUM
    partition).
  - The per-query shifted tables (skew gather) are built by a DRAM round
    trip: qw/qh computed in [q, r] layout on PE, stored to a DRAM scratch,
    re-loaded with an affine skewed access pattern (contiguous 32-element
    runs), then rotated into [table_row, q] layout with DVE 32x32 stream
    transposes.
  - Softmax denominators via an all-ones stationary matmul accumulated over
    k tiles; output normalized once at (d x q) scale, not (k x q).
  - Matmuls run as float32r (full PE column rate at N>=512); fp32 fallback
    available via MM_DT.
"""

import os
import sys

import numpy as np

for _p in ("/opt/trn_rl_repo", "/root/.axon_site/_ro/trn_rl_repo"):
    if os.path.isdir(_p) and _p not in sys.path:
        sys.path.append(_p)

import concourse.bass as bass
import concourse.mybir as mybir
import concourse.tile as tile
from concourse import bacc
from concourse.bass_utils import run_bass_kernel_spmd

B, C, H, W = 16, 512, 32, 32
HW = H * W
NH, DH = 4, 128
SCALE = DH ** -0.5
N_CORES = 8
B_LOC = B // N_CORES

F32 = mybir.dt.float32
F32R = mybir.dt.float32r
BF16 = mybir.dt.bfloat16
FP16 = mybir.dt.float16
MM_DT = F32R  # float32r: full-rate PE; float32: exact but 4x slower
EXP = mybir.ActivationFunctionType.Exp

_CACHE = {}

# f-order permutation: device hw column f = 256*s + 32*qt + j for q = 128*qt + 32*s + j
_QS = np.arange(1024)
_F_OF_Q = (256 * ((_QS % 128) // 32) + 32 * (_QS // 128) + (_QS % 32)).astype(np.int64)


def _mm(ap):
    return ap


def _sel_const():
    """sel[j, kc*128 + p]: j<32 selects shifted width row w2(p)=p%32,
    j>=32 selects shifted height row h2(p)=4*kc+p//32."""
    sel = np.zeros((64, 8 * 128), np.float32)
    for kc in range(8):
        for p in range(128):
            f = 128 * kc + p
            sg, qt, j = f // 256, (f % 256) // 32, f % 32
            q = 128 * qt + 32 * sg + j
            sel[q % 32, kc * 128 + p] = 1.0
            sel[32 + q // 32, kc * 128 + p] = 1.0
    return np.concatenate([sel, sel], axis=0)  # [128, 1024]: both PE row halves


def _emit(tc, nc, xd, wd, reld, seld, onesd, identd, outd, dend, scr_handles):
    from contextlib import ExitStack

    ctx = ExitStack()
    with ctx:
        const = ctx.enter_context(tc.tile_pool(name="const", bufs=1))
        xpool = ctx.enter_context(tc.tile_pool(name="x", bufs=B_LOC))
        qkvp = ctx.enter_context(tc.tile_pool(name="qkv", bufs=3))
        qwhp = ctx.enter_context(tc.tile_pool(name="qwh", bufs=3))
        wvp = ctx.enter_context(tc.tile_pool(name="wv", bufs=3))
        biasp = ctx.enter_context(tc.tile_pool(name="biasv", bufs=3))
        vnatp = ctx.enter_context(tc.tile_pool(name="vnat", bufs=2))
        ptp = ctx.enter_context(tc.tile_pool(name="pt", bufs=3))
        outp = ctx.enter_context(tc.tile_pool(name="outt", bufs=2))
        recp = ctx.enter_context(tc.tile_pool(name="recip", bufs=2))
        # one shared 4-slot pool for all transient [128,512] psum tiles
        ps_misc = ctx.enter_context(tc.tile_pool(name="psmisc", bufs=4, space="PSUM"))
        ps_st = ps_misc
        ps_den = ctx.enter_context(tc.tile_pool(name="psden", bufs=2, space="PSUM"))
        ps_out = ctx.enter_context(tc.tile_pool(name="psout", bufs=2, space="PSUM"))

        # ---- constants / weights ----
        wt_sb = const.tile([128, 4 * 1536], MM_DT, name="wt_sb")
        for kc4 in range(4):
            for h3 in range(3):  # split so the first proj can start early
                nc.sync.dma_start(
                    wt_sb[:, kc4 * 1536 + h3 * 512 : kc4 * 1536 + h3 * 512 + 512],
                    wd[kc4 * 128 : kc4 * 128 + 128, h3 * 512 : h3 * 512 + 512],
                )
        rel_sb = const.tile([128, 256], MM_DT, name="rel_sb")
        nc.gpsimd.dma_start(rel_sb[:], reld)
        sel_sb = const.tile([128, 1024], FP16, name="sel_sb")
        nc.gpsimd.dma_start(sel_sb[:], seld)
        ones_sb = const.tile([128, 128], MM_DT, name="ones_sb")
        nc.gpsimd.dma_start(ones_sb[:], onesd)
        id_sb = const.tile([128, 128], MM_DT, name="id_sb")
        nc.gpsimd.dma_start(id_sb[:], identd)

        x_sb = []

        def _load_x(b):
            xb = xpool.tile([128, 4 * HW], MM_DT, tag="x", name=f"x_sb{b}")
            for kc4 in range(4):
                for qc in range(2):
                    xeng = nc.scalar if (kc4 * 2 + qc) % 2 == 0 else nc.sync
                    xeng.dma_start(
                        xb[:, kc4 * HW + qc * 512 : kc4 * HW + qc * 512 + 512],
                        xd[b][kc4 * 128 : kc4 * 128 + 128, qc * 512 : qc * 512 + 512],
                    )
            x_sb.append(xb)

        _load_x(0)

        def _proj(bn, qkvT, t, col0):
            b = bn // NH
            for qc in range(2):
                ps = ps_misc.tile(
                    [128, 512], F32, tag="misc", name=f"proj{bn}_{t}_{qc}"
                )
                for kc4 in range(4):
                    rhs = (
                        x_sb[b][:, kc4 * HW : kc4 * HW + HW]
                        .rearrange("p (qt sg j) -> p sg qt j", sg=4, j=32)
                        [:, 2 * qc : 2 * qc + 2, :, :]
                    )
                    nc.tensor.matmul(
                        ps[:],
                        _mm(wt_sb[:, kc4 * 1536 + col0 : kc4 * 1536 + col0 + 128]),
                        _mm(rhs),
                        start=(kc4 == 0),
                        stop=(kc4 == 3),
                    )
                if (t + qc) % 2 == 0:
                    nc.vector.tensor_copy(
                        qkvT[:, t * HW + qc * 512 : t * HW + qc * 512 + 512], ps[:]
                    )
                else:
                    nc.scalar.copy(
                        qkvT[:, t * HW + qc * 512 : t * HW + qc * 512 + 512], ps[:]
                    )

        def stage_bias(bn):
            """q projection + rel tables + skew round trip -> (qkvT, bias_vecT).

            Emitted one bn ahead so the DRAM round trip and the DVE stream
            transposes hide under the previous bn's attention matmuls."""
            n = bn % NH
            if bn == NH:
                _load_x(1)
            scr = scr_handles[bn]
            qkvT = qkvp.tile([128, 3 * HW], MM_DT, tag="qkv", name=f"qkvT{bn}")
            _proj(bn, qkvT, 0, n * DH)  # q only

            qwh = qwhp.tile([128, 8 * 126], F32, tag="qwh", name=f"qwh{bn}")
            for qt in range(8):
                ps = ps_misc.tile([128, 256], F32, tag="misc", name=f"qwhp{bn}_{qt}")
                nc.tensor.matmul(
                    ps[:],
                    _mm(qkvT[:, qt * 128 : qt * 128 + 128]),
                    _mm(rel_sb[:]),
                    start=True,
                    stop=True,
                )
                nc.scalar.copy(qwh[:, qt * 126 : qt * 126 + 126], ps[:, :126])

            nc.sync.dma_start(
                scr.ap().rearrange("(a p) r -> p a r", p=128),
                qwh[:].rearrange("p (a r) -> p a r", r=126),
            )
            wv = wvp.tile([128, 256], F32, tag="wv", name=f"wv{bn}")
            hv = wvp.tile([128, 256], F32, tag="hv", name=f"hv{bn}")
            skew_eng = nc.sync if bn < 2 else nc.gpsimd
            for sg in range(4):
                skew_eng.dma_start(
                    wv[32 * sg : 32 * sg + 32, :].rearrange("p (a j) -> p a j", j=32),
                    bass.AP(scr, 31 + 32256 * sg, [[125, 32], [4032, 8], [1, 32]]),
                )
                skew_eng.dma_start(
                    hv[32 * sg : 32 * sg + 32, :].rearrange("p (a j) -> p a j", j=32),
                    bass.AP(scr, 94 + 32255 * sg, [[126, 32], [4028, 8], [1, 32]]),
                )

            bias_f32 = biasp.tile([64, HW], F32, tag="biasf", name=f"biasf{bn}")
            for src, row0 in ((wv, 0), (hv, 32)):
                for sg in range(4):
                    nc.vector.transpose(
                        bias_f32[row0 : row0 + 32, 256 * sg : 256 * sg + 256],
                        src[32 * sg : 32 * sg + 32, :],
                    )
            bias_vecT = biasp.tile([128, HW], FP16, tag="biasv", name=f"biasv{bn}")
            nc.vector.tensor_copy(bias_vecT[0:64, :], bias_f32[:])
            nc.vector.tensor_copy(bias_vecT[64:128, :], bias_f32[:])
            return qkvT, bias_vecT

        def stage_kv(bn, qkvT):
            n = bn % NH
            _proj(bn, qkvT, 1, 512 + n * DH)  # k
            _proj(bn, qkvT, 2, 1024 + n * DH)  # v
            vnat = vnatp.tile([128, HW], MM_DT, tag="vnat", name=f"vnat{bn}")
            for kc in range(8):
                ps = ps_misc.tile([128, 128], MM_DT, tag="misc", name=f"vtr{bn}_{kc}")
                nc.tensor.transpose(
                    ps[:], qkvT[:, 2 * HW + kc * 128 : 2 * HW + kc * 128 + 128], id_sb[:]
                )
                nc.vector.tensor_copy(vnat[:, kc * 128 : kc * 128 + 128], ps[:])
            return vnat

        def stage_attn(bn, qkvT, bias_vecT, vnat):
            b, n = bn // NH, bn % NH

            den_ps = [
                ps_den.tile([1, 512], F32, tag="den", name=f"den{bn}_{i}")
                for i in range(2)
            ]
            out_ps = [
                ps_out.tile([128, 512], F32, tag="out", name=f"outp{bn}_{i}")
                for i in range(2)
            ]
            def _den_pv(pkc, ppT, qc):
                nc.tensor.matmul(
                    den_ps[qc][:],
                    _mm(ones_sb[:, 0:1]),
                    _mm(ppT[:, qc * 512 : qc * 512 + 512]),
                    start=(pkc == 0),
                    stop=(pkc == 7),
                )
                nc.tensor.matmul(
                    out_ps[qc][:],
                    _mm(vnat[:, pkc * 128 : pkc * 128 + 128]),
                    _mm(ppT[:, qc * 512 : qc * 512 + 512]),
                    start=(pkc == 0),
                    stop=(pkc == 7),
                )

            prev = None  # (kc, pT) one step behind: den/pv interleave with
            for kc in range(8):  # the next kc's ST+bias weight loads
                pT = ptp.tile([128, HW], MM_DT, tag="pt", name=f"pt{bn}_{kc}")
                sts = []
                for qc in range(2):
                    st = ps_misc.tile([128, 512], F32, tag="misc", name=f"st{bn}_{kc}_{qc}")
                    sts.append(st)
                    nc.tensor.matmul(
                        st[:],
                        _mm(qkvT[:, HW + kc * 128 : HW + kc * 128 + 128]),
                        _mm(qkvT[:, qc * 512 : qc * 512 + 512]),
                        start=True,
                        stop=False,
                    )
                for qc in range(2):
                    r0 = 64 * qc
                    nc.tensor.matmul(
                        sts[qc][:],
                        _mm(sel_sb[r0 : r0 + 64, kc * 128 : kc * 128 + 128]),
                        _mm(bias_vecT[r0 : r0 + 64, qc * 512 : qc * 512 + 512]),
                        start=False,
                        stop=True,
                    )
                    nc.scalar.activation(
                        pT[:, qc * 512 : qc * 512 + 512], sts[qc][:], EXP
                    )
                for qc in range(2):
                    _den_pv(kc, pT, qc)

            outT = outp.tile([128, HW], F32, tag="outt", name=f"outT{bn}")
            den_sb = recp.tile([1, HW], F32, tag="densb", name=f"densb{bn}")
            for qc in range(2):
                nc.vector.tensor_copy(outT[:, qc * 512 : qc * 512 + 512], out_ps[qc][:])
                nc.scalar.copy(den_sb[:, qc * 512 : qc * 512 + 512], den_ps[qc][:])
            nc.sync.dma_start(outd[b, n], outT[:])
            nc.sync.dma_start(dend[b, n], den_sb[:])

        # software pipeline: bias chains emitted two bn ahead of attention
        n_bn = B_LOC * NH
        from collections import deque

        states = deque([stage_bias(0)])
        kvs = deque([stage_kv(0, states[0][0])])
        states.append(stage_bias(1))
        for bn in range(n_bn):
            if bn + 1 < n_bn:
                kvs.append(stage_kv(bn + 1, states[1][0]))
            if bn + 2 < n_bn:
                states.append(stage_bias(bn + 2))
            qkvT, bias_vecT = states.popleft()
            stage_attn(bn, qkvT, bias_vecT, kvs.popleft())


def _build():
    if "nc" in _CACHE:
        return _CACHE["nc"]
    nc = bacc.Bacc("TRN2", target_bir_lowering=False, debug=False, num_devices=N_CORES)
    xd = nc.dram_tensor("x_r", [B_LOC, C, HW], MM_DT, kind="ExternalInput").ap()
    wd = nc.dram_tensor("w_t", [C, 3 * NH * DH], MM_DT, kind="ExternalInput").ap()
    reld = nc.dram_tensor("rel_t", [128, 256], MM_DT, kind="ExternalInput").ap()
    seld = nc.dram_tensor("sel", [128, 1024], FP16, kind="ExternalInput").ap()
    onesd = nc.dram_tensor("ones", [128, 128], MM_DT, kind="ExternalInput").ap()
    identd = nc.dram_tensor("ident", [128, 128], MM_DT, kind="ExternalInput").ap()
    outd = nc.dram_tensor("out_r", [B_LOC, NH, DH, HW], F32, kind="ExternalOutput").ap()
    dend = nc.dram_tensor("den_r", [B_LOC, NH, 1, HW], F32, kind="ExternalOutput").ap()
    scr_handles = [
        nc.dram_tensor(f"scr{i}", [HW, 126], F32) for i in range(B_LOC * NH)
    ]
    with tile.TileContext(nc) as tc:
        _emit(tc, nc, xd, wd, reld, seld, onesd, identd, outd, dend, scr_handles)
    nc.compile()
    _CACHE["nc"] = nc
    return nc


def _in_maps(x, w_qkv, height_rel, width_rel):
    x = np.ascontiguousarray(np.asarray(x, np.float32))
    w_qkv = np.asarray(w_qkv, np.float32)
    height_rel = np.asarray(height_rel, np.float32)
    width_rel = np.asarray(width_rel, np.float32)

    w_t = np.ascontiguousarray(w_qkv.T)  # [C, 1536]
    w_t[:, 512:1024] *= np.float32(SCALE)  # fold softmax scale into k
    rel_t = np.zeros((128, 256), np.float32)
    rel_t[:, 0:63] = width_rel.T
    rel_t[:, 63:126] = height_rel.T
    sel = _sel_const().astype(np.float16)
    ones = np.ones((128, 128), np.float32)
    ident = np.eye(128, dtype=np.float32)

    shared = {
        "w_t": w_t,
        "rel_t": rel_t,
        "sel": sel,
        "ones": ones,
        "ident": ident,
    }
    maps = []
    for i in range(N_CORES):
        xm = x[i * B_LOC : (i + 1) * B_LOC].reshape(B_LOC, C, HW)
        maps.append({"x_r": np.ascontiguousarray(xm), **shared})
    return maps


def _assemble(results):
    out = np.empty((B, 3 * NH * DH // 3, H, W), np.float32)  # (16, 512, 32, 32)
    for i, r in enumerate(results):
        arr = r["out_r"] / r["den_r"]  # [B_LOC, NH, DH, HW] / [B_LOC, NH, 1, HW]
        arr = arr[..., _F_OF_Q]  # undo the device-side f-ordering of hw columns
        for b in range(B_LOC):
            # flat order of reference output = q*512 + n*128 + d
            out[i * B_LOC + b] = (
                arr[b].transpose(2, 0, 1).reshape(512, 32, 32)
            )
    return out


def run(x, w_qkv, height_rel, width_rel, **spmd_kwargs):
    nc = _build()
    maps = _in_maps(x, w_qkv, height_rel, width_rel)
    res = run_bass_kernel_spmd(nc, maps, core_ids=list(range(N_CORES)), **spmd_kwargs)
    return _assemble(res.results), res


def kernel(x, w_qkv, height_rel, width_rel):
    out, _ = run(x, w_qkv, height_rel, width_rel)
    return out
